# revision 41
# baseline (speedup 1.0000x reference)
"""Multi-head attention forward on 8 Trainium2 NeuronCores.

Sharding: core = (batch b in 0..2, head-group hg in 0..4); each core owns
4 of the 16 heads for one batch element. Q/K/V projections are computed
per-core for its 256 head-dims; attention runs per head with scores kept
transposed (S^T[k, q]); the output projection is row-sharded over W_o,
producing a per-core partial Y that the host sums over the 4 head-groups
of each batch.

v5: the kernel is PE-bound (~850 matmuls, union-busy 85% at the 87%
HAM throttle cap), with ScalarE's exp stream pacing the score ring at
1.34us/k-tile vs the PE's 1.30us: both are attacked:

- Each k-tile's even/odd-head score matmuls write one shared
  [128,2,512] PSUM slot (row-tiled tile_position (0,0)/(64,0)); both
  become ready together when the slot frees, so the PE runs the pair
  concurrently (2 matmuls per 216ns, measured).
- One exp instruction per k-tile covers both heads. Four of every 16
  k-tiles are offloaded from ScalarE to the DVE (the Pool engine
  cannot read PSUM) using a one-instruction Schraudolph exp: i16 =
  round(s*(1024/ln2)*0.125 + 15308) bit-cast as fp16 ~= exp(0.125 s)
  (max rel err ~3.5%, which largely cancels in softmax since numerator
  and denominator use the same approximation). This gives ScalarE
  ~4.7us/block of slack so the PV matmuls never pay the 100ns
  just-in-time semaphore latency on the exp results. The softmax
  normalization multiplies run on the Pool engine (all-SBUF) to make
  DVE room for the extra Schraudolph slices.
- PSUM: scores 2x[128,2,512] (4 banks), projections/output 1x[128,2,512]
  (2 banks), PV ctx accumulators 2x[128,512] (2 banks).
- Score matmuls are emitted under tc.high_priority so the PE always
  prefers feeding the exp engines; PV/projection/output work fills the
  slack. A warm-up burst of junk matmuls releases the PE HAM throttle
  before the first projection.
- Input DMAs alternate between the sync and gpsimd queue engines and
  are all dispatched up front (except the last Q slice, placed late on
  sync) so dispatch serialization never delays the pipeline and never
  blocks gpsimd's normalization broadcasts.
- PV accumulators are copied PSUM->SBUF immediately, putting the 4-hop
  normalization chain off the PSUM-reuse critical path.

All matmul operands are fp16 (PSUM accumulation stays fp32). V is stored
per k-tile in head-pair blocks [V_even | ones | junk | V_odd] (192 cols);
the PV stationary is the 128-wide window at offset 0 (even head: ctx
rows 0:64, denom row 64) or offset 64 (odd head: denom row 0, ctx rows
64:128), so each head's softmax denominator comes free. Weights are
pre-interleaved on the host so their DMAs are contiguous; output is
written fp16 and summed in f32 on the host.
"""

import sys

for _p in ("/opt/trn_rl_repo", "/opt/pypackages"):
    if _p not in sys.path:
        sys.path.append(_p)

from contextlib import ExitStack

import numpy as np

import concourse.bass as bass
import concourse.tile as tile
from concourse import bacc, mybir
from concourse import bass_utils

P = 128
B = 2
S = 2048          # sequence length
D = 1024          # model dim
H = 16            # total heads
DK = 64           # head dim
HL = 4            # heads per core
CL = HL * DK      # local head dims per core (256)
NJ = 4            # 512-wide s-slices
NS = 512
NI = D // P       # 8 contraction tiles over model dim
NK = S // P       # 16 key tiles
VPB = 192         # V pair block: V_even(64) | ones(1) | junk(63) | V_odd(64)
VPAD = 2 * VPB    # 384 cols for 2 head pairs

F32 = mybir.dt.float32
F16 = mybir.dt.float16
I16 = mybir.dt.int16
EXP = mybir.ActivationFunctionType.Exp

# k-tiles whose exp runs on the DVE (Schraudolph) instead of ScalarE
# (the Pool engine cannot read PSUM, so DVE is the only offload target);
# 4/16 along-k coverage keeps the softmax error ~1.2e-2 vs the 2e-2 gate
OFFLOAD_KS = {3, 7, 11, 15}
# i16 = s * (1024/ln2 * 0.125) + (15<<10) + C  ->  bitcast f16 ~ exp(s/8)
SCH_MULT = 184.6650
SCH_ADD = 15308.0


def build_nc():
    nc = bacc.Bacc("TRN2", target_bir_lowering=False, debug=False)

    xqT = nc.dram_tensor("xqT", [D, S], F16, kind="ExternalInput")
    xkT = nc.dram_tensor("xkT", [D, S], F16, kind="ExternalInput")
    xvT = nc.dram_tensor("xvT", [D, S], F16, kind="ExternalInput")
    # weights pre-interleaved on host to [128, n*out] layout
    wqT = nc.dram_tensor("wqT", [P, NI * CL], F16, kind="ExternalInput")
    wkT = nc.dram_tensor("wkT", [P, NI * CL], F16, kind="ExternalInput")
    wvT = nc.dram_tensor("wvT", [P, NI * CL], F16, kind="ExternalInput")
    woT = nc.dram_tensor("woT", [P, (CL // P) * D], F16, kind="ExternalInput")
    y = nc.dram_tensor("y", [S, D], F16, kind="ExternalOutput")

    _dq = [0]

    def dq():
        _dq[0] += 1
        return nc.sync if _dq[0] % 2 == 0 else nc.gpsimd

    with tile.TileContext(nc) as tc, ExitStack() as ctx:
        wpool = ctx.enter_context(tc.tile_pool(name="w", bufs=1))
        big = ctx.enter_context(tc.tile_pool(name="big", bufs=1))
        xpool = ctx.enter_context(tc.tile_pool(name="xs", bufs=26))
        epool = ctx.enter_context(tc.tile_pool(name="ex", bufs=30))
        cpool = ctx.enter_context(tc.tile_pool(name="cs", bufs=4))
        spool = ctx.enter_context(tc.tile_pool(name="sm", bufs=3))
        ypool = ctx.enter_context(tc.tile_pool(name="yo", bufs=2))
        psS = ctx.enter_context(tc.tile_pool(name="psS", bufs=2, space="PSUM"))
        psP = ctx.enter_context(tc.tile_pool(name="psP", bufs=1, space="PSUM"))
        psC = ctx.enter_context(tc.tile_pool(name="psC", bufs=2, space="PSUM"))

        wq_sb = wpool.tile([P, NI, CL], F16)
        wk_sb = wpool.tile([P, NI, CL], F16)
        wv_sb = wpool.tile([P, NI, CL], F16)
        wo_sb = wpool.tile([P, CL // P, D], F16)

        # per-slice K/Q tiles: a score matmul for k-tile k must depend
        # only on the one projection copy that wrote it, not all four
        kT_j = [big.tile([P, 2, NS], F16, tag=f"kT{j}", name=f"kT{j}")
                for j in range(NJ)]
        qT_j = [big.tile([P, 2, NS], F16, tag=f"qT{j}", name=f"qT{j}")
                for j in range(NJ)]
        cT_j = [big.tile([P, 2, NS], F16, tag=f"cT{j}", name=f"cT{j}")
                for j in range(NJ)]
        vt = [big.tile([P, VPAD], F16, tag=f"v{k}", name=f"v{k}")
              for k in range(NK)]

        # ---- PE warm-up: ~4us of junk matmuls releases the HAM gate ---
        wsrc = spool.tile([P, DK], F16, tag="wsrc", name="wsrc")
        nc.vector.memset(wsrc[:], 0.0)
        wsrc2 = spool.tile([P, NS], F16, tag="wsrc2", name="wsrc2")
        nc.vector.memset(wsrc2[:], 0.0)
        warm_ps = psC.tile([P, NS], F32, tag="ctx", name="warm")
        for _ in range(18):
            nc.tensor.matmul(warm_ps[0:DK, 0:DK], wsrc[0:DK, :],
                             wsrc[0:DK, :], start=True, stop=True)

        # ones + junk columns of each V pair block (cols 64:128, 256:320)
        for k in range(NK):
            nc.vector.memset(
                vt[k][:].rearrange("p (b c) -> p b c", c=VPB)[:, :, DK:2 * DK],
                1.0,
            )

        # ---- DMA issue helpers ----------------------------------------
        def dma_w(dst, src):
            dq().dma_start(dst[:].rearrange("p n o -> p (n o)"), src.ap())

        def dma_x(x_dram, jh, eng=None):
            xt = []
            for i in range(NI):
                t = xpool.tile([P, 2, NS], F16, tag="x", name="xt")
                (eng or dq()).dma_start(
                    t[:],
                    x_dram.ap()[i * P:(i + 1) * P, jh * 2 * NS:(jh * 2 + 2) * NS]
                    .rearrange("p (a s) -> p a s", s=NS),
                )
                xt.append(t)
            return xt

        def xti(xt, i):
            return xt[i][:]

        # ---- compute emit helpers -------------------------------------
        def proj_j(xt_jh, w_sb, jj, dst, pool=None):
            # dst <- (X @ W.T)^T for one 512-wide s-slice ([128, 2, 512]);
            # the two ot chains use independent one-bank slots so their
            # copies free PSUM independently
            if pool is None:
                spa = psP.tile([P, NS], F32, tag="ppa", name="pja")
                spb = psP.tile([P, NS], F32, tag="ppb", name="pjb")
                tgt = [spa[:], spb[:]]
            else:
                sp = pool.tile([P, 2, NS], F32, tag="sc", name="pj")
                tgt = [sp[:, 0], sp[:, 1]]
            for i in range(NI):
                for ot in range(2):
                    nc.tensor.matmul(
                        tgt[ot],
                        w_sb[:, i, ot * P:(ot + 1) * P],
                        xti(xt_jh, i)[:, jj],
                        start=(i == 0),
                        stop=(i == NI - 1),
                    )
            if pool is None:
                for ot in range(2):
                    nc.vector.tensor_copy(dst[:, ot], tgt[ot])
            else:
                nc.vector.tensor_copy(dst, sp[:])

        xv = [None, None]

        def vp(c):
            # V-proj chunk c: projects s-tiles 2c, 2c+1 and packs them
            # into vt[2c], vt[2c+1] head-pair blocks. Odd chunks borrow
            # the (still idle) ctx PSUM slots so chunk c+1's matmuls
            # overlap chunk c's pack copies instead of serializing
            # through the single projection slot.
            jh, sbp = divmod(c, 4)
            if c % 2 == 0:
                spa = psP.tile([P, NS], F32, tag="ppa", name="pva")
                spb = psP.tile([P, NS], F32, tag="ppb", name="pvb")
                tgt = [spa[:, 0:CL], spb[:, 0:CL]]
            else:
                ta = psC.tile([P, NS], F32, tag="ctx", name="pva")
                tb = psC.tile([P, NS], F32, tag="ctx", name="pvb")
                tgt = [ta[:, 0:CL], tb[:, 0:CL]]
            for i in range(NI):
                xf = xti(xv[jh], i).rearrange("p a s -> p (a s)")
                for u in range(2):
                    sb = sbp * 2 + u
                    nc.tensor.matmul(
                        tgt[u],
                        xf[:, sb * P:(sb + 1) * P],
                        wv_sb[:, i, :],
                        start=(i == 0),
                        stop=(i == NI - 1),
                    )
            for u in range(2):
                st = 2 * c + u
                vv = vt[st][:].rearrange("p (pr c) -> p pr c", c=VPB)
                pv_ = tgt[u].rearrange("p (pr hc) -> p pr hc", hc=2 * DK)
                nc.vector.tensor_copy(vv[:, :, 0:DK], pv_[:, :, 0:DK])
                nc.vector.tensor_copy(vv[:, :, 2 * DK:3 * DK], pv_[:, :, DK:2 * DK])

        pending = []

        def flush_one():
            fns = pending.pop(0)
            for fn in fns:
                fn()

        def sc_k(ot, j, k, ctx_ps, ctx_sb, off=None):
            # One k-tile: paired even/odd score matmuls into one shared
            # 2-bank slot + a single exp (ScalarE or DVE) for both heads.
            # (Splitting into per-head [128,512] slots/exps was tried and
            # regressed 212->288us: the Act engine charges ~400ns fixed
            # cost per instruction, so two half exps cost 1.66us vs 1.11.)
            sps = psS.tile([P, 2, NS], F32, tag="sc", name="sc")
            # priority just BELOW the PV matmuls (250000): a score pair
            # waiting ~100ns for its slot-free semaphore used to sit at
            # the queue head stalling the PE; behind the PV pair the wait
            # hides under real work (exp is no longer the ring pacer)
            with tc.high_priority(offset=200000):
                for pr in range(2):
                    pr0 = pr * 64
                    nc.tensor.matmul(
                        sps[:, pr],
                        kT_j[k // 4][pr0:pr0 + 64, ot, (k % 4) * P:(k % 4 + 1) * P],
                        qT_j[j][pr0:pr0 + 64, ot, :],
                        start=True,
                        stop=True,
                    )
            ex = epool.tile([P, 2, NS], I16, tag="ex", name="ex")
            if k in (OFFLOAD_KS if off is None else off):
                with tc.high_priority(offset=300):
                    nc.vector.tensor_scalar(
                        ex[:], sps[:], SCH_MULT, SCH_ADD,
                        mybir.AluOpType.mult, mybir.AluOpType.add,
                    )
            else:
                nc.scalar.activation(ex[:].bitcast(F16), sps[:], EXP,
                                     scale=0.125)

            def pv_fn(ex=ex, ot=ot, k=k, first=(k == 0)):
                if first:
                    for pr in range(2):
                        ctx_ps[pr] = psC.tile([P, NS], F32, tag="ctx",
                                              name="ctx")
                exf = ex[:].bitcast(F16)
                # priority between scores and projection/output backlog:
                # late PVs hold ex-pool slots and stall the score ring
                with tc.high_priority(offset=250000):
                    for pr in range(2):
                        vcol = ot * VPB + pr * DK
                        nc.tensor.matmul(
                            ctx_ps[pr][:],
                            vt[k][:, vcol:vcol + P],
                            exf[:, pr],
                            start=(k == 0),
                            stop=(k == NK - 1),
                        )

            fns = [pv_fn]
            if k == NK - 1:
                def cp_fn():
                    # free the PSUM banks; normalize from the SBUF copy
                    for pr in range(2):
                        cs = cpool.tile([P, NS], F32, tag="cs", name="cs")
                        nc.vector.tensor_copy(cs[:], ctx_ps[pr][:])
                        ctx_sb[pr] = cs

                bcs = {}

                def norm_a():
                    # den -> recip -> gpsimd broadcast; the multiply is
                    # deferred one pending entry so the DVE stream never
                    # blocks on the broadcast (the den copy to partition 0
                    # is required: recip at a partition offset diverges on
                    # hardware even though CoreSim accepts it)
                    for pr in range(2):
                        drow = 64 * (1 - pr)
                        cs = ctx_sb[pr]
                        den = spool.tile([1, NS], F32, tag="den", name="den")
                        nc.vector.tensor_copy(den[:], cs[drow:drow + 1, :])
                        rec = spool.tile([1, NS], F32, tag="rec", name="rec")
                        nc.vector.reciprocal_approx_fast(rec[:], den[:])
                        bc = spool.tile([P, NS], F32, tag="bc", name="bc")
                        nc.gpsimd.partition_broadcast(bc[:], rec[:])
                        bcs[pr] = bc

                def norm_b(ot=ot, j=j):
                    # NOT on the Pool engine: gpsimd tensor ops live in a
                    # different microcode library than partition_broadcast,
                    # and the UNLOAD_LIB/LOAD_LIB swap between them stalls
                    # the pipeline for tens of us per block
                    for pr in range(2):
                        pr0 = pr * 64
                        nc.vector.tensor_mul(
                            cT_j[j][pr0:pr0 + 64, ot, :],
                            ctx_sb[pr][pr0:pr0 + 64, :],
                            bcs[pr][pr0:pr0 + 64, :],
                        )
                fns.extend([cp_fn, norm_a])
                pending.append(fns)
                pending.append([norm_b])
                return
            pending.append(fns)

        def emit_y_qb(j, qb, tail=False):
            # tail mode: odd chains borrow the (now idle) ctx slots so
            # the last four output chains don't serialize through the
            # single projection slot
            ysb = ypool.tile([P, D], F16, tag="y", name="ysb")
            yv = ysb[:].rearrange("p (a s) -> p a s", s=NS)
            if tail and qb % 2 == 1:
                ta = psC.tile([P, NS], F32, tag="ctx", name="ypa")
                tb = psC.tile([P, NS], F32, tag="ctx", name="ypb")
                tgt = [ta[:], tb[:]]
            else:
                ypa = psP.tile([P, NS], F32, tag="ppa", name="ya")
                ypb = psP.tile([P, NS], F32, tag="ppb", name="yb")
                tgt = [ypa[:], ypb[:]]
            for ct in range(2):
                for oh in range(2):
                    nc.tensor.matmul(
                        tgt[oh],
                        cT_j[j][:, ct, qb * P:(qb + 1) * P],
                        wo_sb[:, ct, oh * NS:(oh + 1) * NS],
                        start=(ct == 0),
                        stop=(ct == 1),
                    )
            for oh in range(2):
                nc.vector.tensor_copy(yv[:, oh], tgt[oh])
            nc.sync.dma_start(
                y.ap()[(j * 4 + qb) * P:(j * 4 + qb + 1) * P, :], ysb[:]
            )

        def emit_block(ot, j, lag=4, weave_y=None):
            # weave_y: output chains for q-slice weave_y are interleaved
            # into this block's k-loop so they overlap the attention
            # stream instead of serializing at the kernel tail (their cT
            # inputs are two block-norms old by then)
            ctx_ps, ctx_sb = {}, {}
            for k in range(NK):
                sc_k(ot, j, k, ctx_ps, ctx_sb)
                while lag is not None and len(pending) > lag:
                    flush_one()
                # y chains woven 4 tiles apart (after this tile's PV
                # flush) so each qb's psP banks are long free when the
                # next qb needs them and the Act engine keeps receiving
                # scores at a steady cadence instead of burst-then-idle
                if weave_y is not None and k in (2, 6, 10, 14):
                    emit_y_qb(weave_y, (k - 2) // 4)

        def emit_y(j, tail=False):
            for qb in range(4):
                emit_y_qb(j, qb, tail)

        # ---- pipelined schedule ---------------------------------------
        dma_w(wq_sb, wqT)
        dma_w(wk_sb, wkT)
        xq0 = dma_x(xqT, 0)
        xk0 = dma_x(xkT, 0)
        xk1 = dma_x(xkT, 1)
        dma_w(wv_sb, wvT)
        xv[0] = dma_x(xvT, 0)
        dma_w(wo_sb, woT)
        xv[1] = dma_x(xvT, 1)

        # first Q/K chains borrow the (still idle) score slots so the
        # single psP slot doesn't serialize the pipeline start
        # second warm-up burst first in line: N=512 junk matmuls span the
        # input-DMA wait so the HAM MID window never sees a >3.4us idle
        warm2 = psC.tile([P, NS], F32, tag="ctx", name="warm2")
        for _ in range(13):
            nc.tensor.matmul(warm2[0:DK, :], wsrc[0:DK, :],
                             wsrc2[0:DK, :], start=True, stop=True)

        proj_j(xq0, wq_sb, 0, qT_j[0][:, :, :], pool=psS)
        proj_j(xk0, wk_sb, 0, kT_j[0][:, :, :], pool=psS)
        proj_j(xk0, wk_sb, 1, kT_j[1][:, :, :])
        proj_j(xk1, wk_sb, 0, kT_j[2][:, :, :])
        proj_j(xk1, wk_sb, 1, kT_j[3][:, :, :])

        # block (0,0) fills the pipeline with nothing else for ScalarE to
        # overlap, so split its exps 50/50 with the (idle) DVE; the extra
        # Schraudolph tiles on 2 of 64 block-instances are negligible
        b00_ctx, b00_cs = {}, {}
        for k in range(NK):
            sc_k(0, 0, k, b00_ctx, b00_cs,      # no flush: V not built yet
                 off={1, 3, 5, 7, 9, 11, 13, 15})

        proj_j(xq0, wq_sb, 1, qT_j[1][:, :, :])

        # block (1,0) woven with V-proj chunks; flush (0,0) PV entries as
        # soon as the vt tiles they read exist (entry m needs vp(m//2)).
        b10_ctx, b10_cs = {}, {}
        for k in range(NK):
            sc_k(1, 0, k, b10_ctx, b10_cs)
            if k < 8:
                vp(k)
        # flush (0,0) PVs only now: the odd V chunks borrow ctx PSUM
        # slots, so ctx allocations must follow all vp allocations
        for _ in range(16):
            flush_one()

        xq1 = dma_x(xqT, 1, eng=nc.sync)        # late; sync is free now
        proj_j(xq1, wq_sb, 0, qT_j[2][:, :, :])
        proj_j(xq1, wq_sb, 1, qT_j[3][:, :, :])

        emit_block(0, 1)
        emit_block(1, 1)
        emit_block(0, 2, weave_y=0)
        emit_block(1, 2)
        emit_block(0, 3, weave_y=1)
        emit_block(1, 3, weave_y=2)
        while pending:
            flush_one()
        # keep-hot junk: the final norm chain leaves the PE idle for ~2us,
        # which drops it to the MID p-state and makes the last 16 output
        # matmuls run ~3x slow; these fillers keep the clock up
        warm3 = psS.tile([P, 2, NS], F32, tag="sc", name="warm3")
        for _ in range(6):
            nc.tensor.matmul(warm3[0:DK, 0], wsrc[0:DK, :],
                             wsrc2[0:DK, :], start=True, stop=True)
        emit_y(3, tail=True)

    nc.compile()
    return nc


_NC = None


def _get_nc():
    global _NC
    if _NC is None:
        _NC = build_nc()
    return _NC


def _interleave_w(w):
    # [NI*P, O] -> [P, NI*O] so the SBUF load DMA is contiguous
    n = w.shape[0] // P
    return np.ascontiguousarray(
        w.reshape(n, P, w.shape[1]).transpose(1, 0, 2).reshape(P, -1)
    ).astype(np.float16)


def _shard_inputs(Query, Key, Value, W_q, W_k, W_v, W_o):
    in_maps = []
    xT = {}
    for b in range(B):
        xT[b] = (
            np.ascontiguousarray(Query[b].T).astype(np.float16),
            np.ascontiguousarray(Key[b].T).astype(np.float16),
            np.ascontiguousarray(Value[b].T).astype(np.float16),
        )
    for b in range(B):
        for hg in range(4):
            r0 = hg * CL
            in_maps.append({
                "xqT": xT[b][0],
                "xkT": xT[b][1],
                "xvT": xT[b][2],
                "wqT": _interleave_w(np.ascontiguousarray(W_q[r0:r0 + CL, :].T)),
                "wkT": _interleave_w(np.ascontiguousarray(W_k[r0:r0 + CL, :].T)),
                "wvT": _interleave_w(np.ascontiguousarray(W_v[r0:r0 + CL, :].T)),
                "woT": _interleave_w(np.ascontiguousarray(W_o[:, r0:r0 + CL].T)),
            })
    return in_maps


def _reference_np(Query, Key, Value, mask, W_q, W_k, W_v, W_o):
    # Fallback for a non-trivial mask (never hit for the spec'd inputs).
    out = np.empty((B, S, D), dtype=np.float32)
    m = np.broadcast_to(mask, (1, 1, S, S))[0, 0]
    for b in range(B):
        Q = (Query[b] @ W_q.T).reshape(S, H, DK).transpose(1, 0, 2)
        K = (Key[b] @ W_k.T).reshape(S, H, DK).transpose(1, 0, 2)
        V = (Value[b] @ W_v.T).reshape(S, H, DK).transpose(1, 0, 2)
        ctx = np.empty((H, S, DK), dtype=np.float32)
        for h in range(H):
            s = (Q[h] @ K[h].T) / np.sqrt(DK)
            s = np.where(m == 0, -1e9, s)
            s -= s.max(axis=-1, keepdims=True)
            e = np.exp(s)
            ctx[h] = (e / e.sum(axis=-1, keepdims=True)) @ V[h]
        out[b] = ctx.transpose(1, 0, 2).reshape(S, D) @ W_o.T
    return out


def kernel(Query, Key, Value, mask, W_q, W_k, W_v, W_o, **_ignored):
    Query = np.asarray(Query, dtype=np.float32)
    Key = np.asarray(Key, dtype=np.float32)
    Value = np.asarray(Value, dtype=np.float32)
    W_q = np.asarray(W_q, dtype=np.float32)
    W_k = np.asarray(W_k, dtype=np.float32)
    W_v = np.asarray(W_v, dtype=np.float32)
    W_o = np.asarray(W_o, dtype=np.float32)

    if not np.all(np.asarray(mask) != 0):
        return _reference_np(Query, Key, Value, np.asarray(mask),
                             W_q, W_k, W_v, W_o)

    nc = _get_nc()
    in_maps = _shard_inputs(Query, Key, Value, W_q, W_k, W_v, W_o)
    res = bass_utils.run_bass_kernel_spmd(nc, in_maps, core_ids=list(range(8)))
    out = np.zeros((B, S, D), dtype=np.float32)
    for b in range(B):
        for hg in range(4):
            out[b] += res.results[b * 4 + hg]["y"].astype(np.float32)
    return out



# revision 43
# speedup vs baseline: 1.0590x; 1.0590x over previous
"""Multi-head attention forward on 8 Trainium2 NeuronCores.

Sharding: core = (batch b in 0..2, head-group hg in 0..4); each core owns
4 of the 16 heads for one batch element. Q/K/V projections are computed
per-core for its 256 head-dims; attention runs per head with scores kept
transposed (S^T[k, q]); the output projection is row-sharded over W_o,
producing a per-core partial Y that the host sums over the 4 head-groups
of each batch.

v5: the kernel is PE-bound (~850 matmuls, union-busy 85% at the 87%
HAM throttle cap), with ScalarE's exp stream pacing the score ring at
1.34us/k-tile vs the PE's 1.30us: both are attacked:

- Each k-tile's even/odd-head score matmuls write one shared
  [128,2,512] PSUM slot (row-tiled tile_position (0,0)/(64,0)); both
  become ready together when the slot frees, so the PE runs the pair
  concurrently (2 matmuls per 216ns, measured).
- One exp instruction per k-tile covers both heads. Four of every 16
  k-tiles are offloaded from ScalarE to the DVE (the Pool engine
  cannot read PSUM) using a one-instruction Schraudolph exp: i16 =
  round(s*(1024/ln2)*0.125 + 15308) bit-cast as fp16 ~= exp(0.125 s)
  (max rel err ~3.5%, which largely cancels in softmax since numerator
  and denominator use the same approximation). This gives ScalarE
  ~4.7us/block of slack so the PV matmuls never pay the 100ns
  just-in-time semaphore latency on the exp results. The softmax
  normalization multiplies run on the Pool engine (all-SBUF) to make
  DVE room for the extra Schraudolph slices.
- PSUM: scores 2x[128,2,512] (4 banks), projections/output 1x[128,2,512]
  (2 banks), PV ctx accumulators 2x[128,512] (2 banks).
- Score matmuls are emitted under tc.high_priority so the PE always
  prefers feeding the exp engines; PV/projection/output work fills the
  slack. A warm-up burst of junk matmuls releases the PE HAM throttle
  before the first projection.
- Input DMAs alternate between the sync and gpsimd queue engines and
  are all dispatched up front (except the last Q slice, placed late on
  sync) so dispatch serialization never delays the pipeline and never
  blocks gpsimd's normalization broadcasts.
- PV accumulators are copied PSUM->SBUF immediately, putting the 4-hop
  normalization chain off the PSUM-reuse critical path.

All matmul operands are fp16 (PSUM accumulation stays fp32). V is stored
per k-tile in head-pair blocks [V_even | ones | junk | V_odd] (192 cols);
the PV stationary is the 128-wide window at offset 0 (even head: ctx
rows 0:64, denom row 64) or offset 64 (odd head: denom row 0, ctx rows
64:128), so each head's softmax denominator comes free. Weights are
pre-interleaved on the host so their DMAs are contiguous; output is
written fp16 and summed in f32 on the host.
"""

import sys

for _p in ("/opt/trn_rl_repo", "/opt/pypackages"):
    if _p not in sys.path:
        sys.path.append(_p)

from contextlib import ExitStack

import numpy as np

import concourse.bass as bass
import concourse.tile as tile
from concourse import bacc, mybir
from concourse import bass_utils

P = 128
B = 2
S = 2048          # sequence length
D = 1024          # model dim
H = 16            # total heads
DK = 64           # head dim
HL = 4            # heads per core
CL = HL * DK      # local head dims per core (256)
NJ = 4            # 512-wide s-slices
NS = 512
NI = D // P       # 8 contraction tiles over model dim
NK = S // P       # 16 key tiles
VPB = 192         # V pair block: V_even(64) | ones(1) | junk(63) | V_odd(64)
VPAD = 2 * VPB    # 384 cols for 2 head pairs

F32 = mybir.dt.float32
F16 = mybir.dt.float16
I16 = mybir.dt.int16
EXP = mybir.ActivationFunctionType.Exp

# k-tiles whose exp runs on the DVE (Schraudolph) instead of ScalarE
# (the Pool engine cannot read PSUM, so DVE is the only offload target);
# 4/16 along-k coverage keeps the softmax error ~1.2e-2 vs the 2e-2 gate
OFFLOAD_KS = {2, 5, 8, 10, 13, 15}
# i16 = s * (1024/ln2 * 0.125) + (15<<10) + C  ->  bitcast f16 ~ exp(s/8)
SCH_MULT = 184.6650
SCH_ADD = 15308.0


def build_nc():
    nc = bacc.Bacc("TRN2", target_bir_lowering=False, debug=False)

    xqT = nc.dram_tensor("xqT", [D, S], F16, kind="ExternalInput")
    xkT = nc.dram_tensor("xkT", [D, S], F16, kind="ExternalInput")
    xvT = nc.dram_tensor("xvT", [D, S], F16, kind="ExternalInput")
    # weights pre-interleaved on host to [128, n*out] layout
    wqT = nc.dram_tensor("wqT", [P, NI * CL], F16, kind="ExternalInput")
    wkT = nc.dram_tensor("wkT", [P, NI * CL], F16, kind="ExternalInput")
    wvT = nc.dram_tensor("wvT", [P, NI * CL], F16, kind="ExternalInput")
    woT = nc.dram_tensor("woT", [P, (CL // P) * D], F16, kind="ExternalInput")
    y = nc.dram_tensor("y", [S, D], F16, kind="ExternalOutput")

    _dq = [0]

    def dq():
        _dq[0] += 1
        return nc.sync if _dq[0] % 2 == 0 else nc.gpsimd

    with tile.TileContext(nc) as tc, ExitStack() as ctx:
        wpool = ctx.enter_context(tc.tile_pool(name="w", bufs=1))
        big = ctx.enter_context(tc.tile_pool(name="big", bufs=1))
        xpool = ctx.enter_context(tc.tile_pool(name="xs", bufs=26))
        epool = ctx.enter_context(tc.tile_pool(name="ex", bufs=30))
        cpool = ctx.enter_context(tc.tile_pool(name="cs", bufs=4))
        spool = ctx.enter_context(tc.tile_pool(name="sm", bufs=3))
        ypool = ctx.enter_context(tc.tile_pool(name="yo", bufs=2))
        psS = ctx.enter_context(tc.tile_pool(name="psS", bufs=2, space="PSUM"))
        psP = ctx.enter_context(tc.tile_pool(name="psP", bufs=1, space="PSUM"))
        psC = ctx.enter_context(tc.tile_pool(name="psC", bufs=2, space="PSUM"))

        wq_sb = wpool.tile([P, NI, CL], F16)
        wk_sb = wpool.tile([P, NI, CL], F16)
        wv_sb = wpool.tile([P, NI, CL], F16)
        wo_sb = wpool.tile([P, CL // P, D], F16)

        # per-slice K/Q tiles: a score matmul for k-tile k must depend
        # only on the one projection copy that wrote it, not all four
        kT_j = [big.tile([P, 2, NS], F16, tag=f"kT{j}", name=f"kT{j}")
                for j in range(NJ)]
        qT_j = [big.tile([P, 2, NS], F16, tag=f"qT{j}", name=f"qT{j}")
                for j in range(NJ)]
        cT_j = [big.tile([P, 2, NS], F16, tag=f"cT{j}", name=f"cT{j}")
                for j in range(NJ)]
        vt = [big.tile([P, VPAD], F16, tag=f"v{k}", name=f"v{k}")
              for k in range(NK)]

        # ---- PE warm-up: ~4us of junk matmuls releases the HAM gate ---
        wsrc = spool.tile([P, DK], F16, tag="wsrc", name="wsrc")
        nc.vector.memset(wsrc[:], 0.0)
        wsrc2 = spool.tile([P, NS], F16, tag="wsrc2", name="wsrc2")
        nc.vector.memset(wsrc2[:], 0.0)
        warm_ps = psC.tile([P, NS], F32, tag="ctx", name="warm")
        for _ in range(18):
            nc.tensor.matmul(warm_ps[0:DK, 0:DK], wsrc[0:DK, :],
                             wsrc[0:DK, :], start=True, stop=True)

        # ones + junk columns of each V pair block (cols 64:128, 256:320)
        for k in range(NK):
            nc.vector.memset(
                vt[k][:].rearrange("p (b c) -> p b c", c=VPB)[:, :, DK:2 * DK],
                1.0,
            )

        # ---- DMA issue helpers ----------------------------------------
        def dma_w(dst, src):
            dq().dma_start(dst[:].rearrange("p n o -> p (n o)"), src.ap())

        def dma_x(x_dram, jh, eng=None):
            xt = []
            for i in range(NI):
                t = xpool.tile([P, 2, NS], F16, tag="x", name="xt")
                (eng or dq()).dma_start(
                    t[:],
                    x_dram.ap()[i * P:(i + 1) * P, jh * 2 * NS:(jh * 2 + 2) * NS]
                    .rearrange("p (a s) -> p a s", s=NS),
                )
                xt.append(t)
            return xt

        def xti(xt, i):
            return xt[i][:]

        # ---- compute emit helpers -------------------------------------
        def proj_j(xt_jh, w_sb, jj, dst, pool=None):
            # dst <- (X @ W.T)^T for one 512-wide s-slice ([128, 2, 512]);
            # the two ot chains use independent one-bank slots so their
            # copies free PSUM independently
            if pool is None:
                spa = psP.tile([P, NS], F32, tag="ppa", name="pja")
                spb = psP.tile([P, NS], F32, tag="ppb", name="pjb")
                tgt = [spa[:], spb[:]]
            else:
                sp = pool.tile([P, 2, NS], F32, tag="sc", name="pj")
                tgt = [sp[:, 0], sp[:, 1]]
            for i in range(NI):
                for ot in range(2):
                    nc.tensor.matmul(
                        tgt[ot],
                        w_sb[:, i, ot * P:(ot + 1) * P],
                        xti(xt_jh, i)[:, jj],
                        start=(i == 0),
                        stop=(i == NI - 1),
                    )
            if pool is None:
                for ot in range(2):
                    nc.vector.tensor_copy(dst[:, ot], tgt[ot])
            else:
                nc.vector.tensor_copy(dst, sp[:])

        xv = [None, None]

        def vp(c):
            # V-proj chunk c: projects s-tiles 2c, 2c+1 and packs them
            # into vt[2c], vt[2c+1] head-pair blocks. Odd chunks borrow
            # the (still idle) ctx PSUM slots so chunk c+1's matmuls
            # overlap chunk c's pack copies instead of serializing
            # through the single projection slot.
            jh, sbp = divmod(c, 4)
            if c % 2 == 0:
                spa = psP.tile([P, NS], F32, tag="ppa", name="pva")
                spb = psP.tile([P, NS], F32, tag="ppb", name="pvb")
                tgt = [spa[:, 0:CL], spb[:, 0:CL]]
            else:
                ta = psC.tile([P, NS], F32, tag="ctx", name="pva")
                tb = psC.tile([P, NS], F32, tag="ctx", name="pvb")
                tgt = [ta[:, 0:CL], tb[:, 0:CL]]
            for i in range(NI):
                xf = xti(xv[jh], i).rearrange("p a s -> p (a s)")
                for u in range(2):
                    sb = sbp * 2 + u
                    nc.tensor.matmul(
                        tgt[u],
                        xf[:, sb * P:(sb + 1) * P],
                        wv_sb[:, i, :],
                        start=(i == 0),
                        stop=(i == NI - 1),
                    )
            for u in range(2):
                st = 2 * c + u
                vv = vt[st][:].rearrange("p (pr c) -> p pr c", c=VPB)
                pv_ = tgt[u].rearrange("p (pr hc) -> p pr hc", hc=2 * DK)
                nc.vector.tensor_copy(vv[:, :, 0:DK], pv_[:, :, 0:DK])
                nc.vector.tensor_copy(vv[:, :, 2 * DK:3 * DK], pv_[:, :, DK:2 * DK])

        pending = []

        def flush_one():
            fns = pending.pop(0)
            for fn in fns:
                fn()

        def sc_k(ot, j, k, ctx_ps, ctx_sb, off=None):
            # One k-tile: paired even/odd score matmuls into one shared
            # 2-bank slot + a single exp (ScalarE or DVE) for both heads.
            # (Splitting into per-head [128,512] slots/exps was tried and
            # regressed 212->288us: the Act engine charges ~400ns fixed
            # cost per instruction, so two half exps cost 1.66us vs 1.11.)
            sps = psS.tile([P, 2, NS], F32, tag="sc", name="sc")
            with tc.high_priority(offset=500000):
                for pr in range(2):
                    pr0 = pr * 64
                    nc.tensor.matmul(
                        sps[:, pr],
                        kT_j[k // 4][pr0:pr0 + 64, ot, (k % 4) * P:(k % 4 + 1) * P],
                        qT_j[j][pr0:pr0 + 64, ot, :],
                        start=True,
                        stop=True,
                    )
            ex = epool.tile([P, 2, NS], I16, tag="ex", name="ex")
            if k in (OFFLOAD_KS if off is None else off):
                with tc.high_priority(offset=300):
                    nc.vector.tensor_scalar(
                        ex[:], sps[:], SCH_MULT, SCH_ADD,
                        mybir.AluOpType.mult, mybir.AluOpType.add,
                    )
            else:
                nc.scalar.activation(ex[:].bitcast(F16), sps[:], EXP,
                                     scale=0.125)

            def pv_fn(ex=ex, ot=ot, k=k, first=(k == 0)):
                if first:
                    for pr in range(2):
                        ctx_ps[pr] = psC.tile([P, NS], F32, tag="ctx",
                                              name="ctx")
                exf = ex[:].bitcast(F16)
                # priority between scores and projection/output backlog:
                # late PVs hold ex-pool slots and stall the score ring
                with tc.high_priority(offset=250000):
                    for pr in range(2):
                        vcol = ot * VPB + pr * DK
                        nc.tensor.matmul(
                            ctx_ps[pr][:],
                            vt[k][:, vcol:vcol + P],
                            exf[:, pr],
                            start=(k == 0),
                            stop=(k == NK - 1),
                        )

            fns = [pv_fn]
            if k == NK - 1:
                def cp_fn():
                    # free the PSUM banks; normalize from the SBUF copy
                    for pr in range(2):
                        cs = cpool.tile([P, NS], F32, tag="cs", name="cs")
                        nc.vector.tensor_copy(cs[:], ctx_ps[pr][:])
                        ctx_sb[pr] = cs

                bcs = {}

                def norm_a():
                    # den -> recip -> gpsimd broadcast; the multiply is
                    # deferred one pending entry so the DVE stream never
                    # blocks on the broadcast (the den copy to partition 0
                    # is required: recip at a partition offset diverges on
                    # hardware even though CoreSim accepts it)
                    for pr in range(2):
                        drow = 64 * (1 - pr)
                        cs = ctx_sb[pr]
                        den = spool.tile([1, NS], F32, tag="den", name="den")
                        nc.vector.tensor_copy(den[:], cs[drow:drow + 1, :])
                        rec = spool.tile([1, NS], F32, tag="rec", name="rec")
                        nc.vector.reciprocal_approx_fast(rec[:], den[:])
                        bc = spool.tile([P, NS], F32, tag="bc", name="bc")
                        nc.gpsimd.partition_broadcast(bc[:], rec[:])
                        bcs[pr] = bc

                def norm_b(ot=ot, j=j):
                    # NOT on the Pool engine: gpsimd tensor ops live in a
                    # different microcode library than partition_broadcast,
                    # and the UNLOAD_LIB/LOAD_LIB swap between them stalls
                    # the pipeline for tens of us per block
                    for pr in range(2):
                        pr0 = pr * 64
                        nc.vector.tensor_mul(
                            cT_j[j][pr0:pr0 + 64, ot, :],
                            ctx_sb[pr][pr0:pr0 + 64, :],
                            bcs[pr][pr0:pr0 + 64, :],
                        )
                fns.extend([cp_fn, norm_a])
                pending.append(fns)
                pending.append([norm_b])
                return
            pending.append(fns)

        def emit_y_qb(j, qb, tail=False):
            # tail mode: odd chains borrow the (now idle) ctx slots so
            # the last four output chains don't serialize through the
            # single projection slot
            ysb = ypool.tile([P, D], F16, tag="y", name="ysb")
            yv = ysb[:].rearrange("p (a s) -> p a s", s=NS)
            if tail and qb % 2 == 1:
                ta = psC.tile([P, NS], F32, tag="ctx", name="ypa")
                tb = psC.tile([P, NS], F32, tag="ctx", name="ypb")
                tgt = [ta[:], tb[:]]
            else:
                ypa = psP.tile([P, NS], F32, tag="ppa", name="ya")
                ypb = psP.tile([P, NS], F32, tag="ppb", name="yb")
                tgt = [ypa[:], ypb[:]]
            for ct in range(2):
                for oh in range(2):
                    nc.tensor.matmul(
                        tgt[oh],
                        cT_j[j][:, ct, qb * P:(qb + 1) * P],
                        wo_sb[:, ct, oh * NS:(oh + 1) * NS],
                        start=(ct == 0),
                        stop=(ct == 1),
                    )
            for oh in range(2):
                nc.vector.tensor_copy(yv[:, oh], tgt[oh])
            nc.sync.dma_start(
                y.ap()[(j * 4 + qb) * P:(j * 4 + qb + 1) * P, :], ysb[:]
            )

        def emit_block(ot, j, lag=4, weave_y=None):
            # weave_y: output chains for q-slice weave_y are interleaved
            # into this block's k-loop so they overlap the attention
            # stream instead of serializing at the kernel tail (their cT
            # inputs are two block-norms old by then)
            ctx_ps, ctx_sb = {}, {}
            for k in range(NK):
                sc_k(ot, j, k, ctx_ps, ctx_sb)
                while lag is not None and len(pending) > lag:
                    flush_one()
                # y chains woven 4 tiles apart (after this tile's PV
                # flush) so each qb's psP banks are long free when the
                # next qb needs them and the Act engine keeps receiving
                # scores at a steady cadence instead of burst-then-idle
                if weave_y is not None and k in (2, 6, 10, 14):
                    emit_y_qb(weave_y, (k - 2) // 4)

        def emit_y(j, tail=False):
            for qb in range(4):
                emit_y_qb(j, qb, tail)

        # ---- pipelined schedule ---------------------------------------
        dma_w(wq_sb, wqT)
        dma_w(wk_sb, wkT)
        xq0 = dma_x(xqT, 0)
        xk0 = dma_x(xkT, 0)
        xk1 = dma_x(xkT, 1)
        dma_w(wv_sb, wvT)
        xv[0] = dma_x(xvT, 0)
        dma_w(wo_sb, woT)
        xv[1] = dma_x(xvT, 1)

        # first Q/K chains borrow the (still idle) score slots so the
        # single psP slot doesn't serialize the pipeline start
        # second warm-up burst first in line: N=512 junk matmuls span the
        # input-DMA wait so the HAM MID window never sees a >3.4us idle
        warm2 = psC.tile([P, NS], F32, tag="ctx", name="warm2")
        for _ in range(13):
            nc.tensor.matmul(warm2[0:DK, :], wsrc[0:DK, :],
                             wsrc2[0:DK, :], start=True, stop=True)

        proj_j(xq0, wq_sb, 0, qT_j[0][:, :, :], pool=psS)
        proj_j(xk0, wk_sb, 0, kT_j[0][:, :, :], pool=psS)
        proj_j(xk0, wk_sb, 1, kT_j[1][:, :, :])
        proj_j(xk1, wk_sb, 0, kT_j[2][:, :, :])
        proj_j(xk1, wk_sb, 1, kT_j[3][:, :, :])

        # block (0,0) fills the pipeline with nothing else for ScalarE to
        # overlap, so split its exps 50/50 with the (idle) DVE; the extra
        # Schraudolph tiles on 2 of 64 block-instances are negligible
        b00_ctx, b00_cs = {}, {}
        for k in range(NK):
            sc_k(0, 0, k, b00_ctx, b00_cs,      # no flush: V not built yet
                 off={1, 3, 5, 7, 9, 11, 13, 15})

        proj_j(xq0, wq_sb, 1, qT_j[1][:, :, :])

        # block (1,0) woven with V-proj chunks; flush (0,0) PV entries as
        # soon as the vt tiles they read exist (entry m needs vp(m//2)).
        b10_ctx, b10_cs = {}, {}
        for k in range(NK):
            sc_k(1, 0, k, b10_ctx, b10_cs)
            if k < 8:
                vp(k)
        # flush (0,0) PVs only now: the odd V chunks borrow ctx PSUM
        # slots, so ctx allocations must follow all vp allocations
        for _ in range(16):
            flush_one()

        xq1 = dma_x(xqT, 1, eng=nc.sync)        # late; sync is free now
        proj_j(xq1, wq_sb, 0, qT_j[2][:, :, :])
        proj_j(xq1, wq_sb, 1, qT_j[3][:, :, :])

        emit_block(0, 1)
        emit_block(1, 1)
        emit_block(0, 2, weave_y=0)
        emit_block(1, 2)
        emit_block(0, 3, weave_y=1)
        emit_block(1, 3, weave_y=2)
        while pending:
            flush_one()
        # keep-hot junk: the final norm chain leaves the PE idle for ~2us,
        # which drops it to the MID p-state and makes the last 16 output
        # matmuls run ~3x slow; these fillers keep the clock up
        warm3 = psS.tile([P, 2, NS], F32, tag="sc", name="warm3")
        for _ in range(6):
            nc.tensor.matmul(warm3[0:DK, 0], wsrc[0:DK, :],
                             wsrc2[0:DK, :], start=True, stop=True)
        emit_y(3, tail=True)

    nc.compile()
    return nc


_NC = None


def _get_nc():
    global _NC
    if _NC is None:
        _NC = build_nc()
    return _NC


def _interleave_w(w):
    # [NI*P, O] -> [P, NI*O] so the SBUF load DMA is contiguous
    n = w.shape[0] // P
    return np.ascontiguousarray(
        w.reshape(n, P, w.shape[1]).transpose(1, 0, 2).reshape(P, -1)
    ).astype(np.float16)


def _shard_inputs(Query, Key, Value, W_q, W_k, W_v, W_o):
    in_maps = []
    xT = {}
    for b in range(B):
        xT[b] = (
            np.ascontiguousarray(Query[b].T).astype(np.float16),
            np.ascontiguousarray(Key[b].T).astype(np.float16),
            np.ascontiguousarray(Value[b].T).astype(np.float16),
        )
    for b in range(B):
        for hg in range(4):
            r0 = hg * CL
            in_maps.append({
                "xqT": xT[b][0],
                "xkT": xT[b][1],
                "xvT": xT[b][2],
                "wqT": _interleave_w(np.ascontiguousarray(W_q[r0:r0 + CL, :].T)),
                "wkT": _interleave_w(np.ascontiguousarray(W_k[r0:r0 + CL, :].T)),
                "wvT": _interleave_w(np.ascontiguousarray(W_v[r0:r0 + CL, :].T)),
                "woT": _interleave_w(np.ascontiguousarray(W_o[:, r0:r0 + CL].T)),
            })
    return in_maps


def _reference_np(Query, Key, Value, mask, W_q, W_k, W_v, W_o):
    # Fallback for a non-trivial mask (never hit for the spec'd inputs).
    out = np.empty((B, S, D), dtype=np.float32)
    m = np.broadcast_to(mask, (1, 1, S, S))[0, 0]
    for b in range(B):
        Q = (Query[b] @ W_q.T).reshape(S, H, DK).transpose(1, 0, 2)
        K = (Key[b] @ W_k.T).reshape(S, H, DK).transpose(1, 0, 2)
        V = (Value[b] @ W_v.T).reshape(S, H, DK).transpose(1, 0, 2)
        ctx = np.empty((H, S, DK), dtype=np.float32)
        for h in range(H):
            s = (Q[h] @ K[h].T) / np.sqrt(DK)
            s = np.where(m == 0, -1e9, s)
            s -= s.max(axis=-1, keepdims=True)
            e = np.exp(s)
            ctx[h] = (e / e.sum(axis=-1, keepdims=True)) @ V[h]
        out[b] = ctx.transpose(1, 0, 2).reshape(S, D) @ W_o.T
    return out


def kernel(Query, Key, Value, mask, W_q, W_k, W_v, W_o, **_ignored):
    Query = np.asarray(Query, dtype=np.float32)
    Key = np.asarray(Key, dtype=np.float32)
    Value = np.asarray(Value, dtype=np.float32)
    W_q = np.asarray(W_q, dtype=np.float32)
    W_k = np.asarray(W_k, dtype=np.float32)
    W_v = np.asarray(W_v, dtype=np.float32)
    W_o = np.asarray(W_o, dtype=np.float32)

    if not np.all(np.asarray(mask) != 0):
        return _reference_np(Query, Key, Value, np.asarray(mask),
                             W_q, W_k, W_v, W_o)

    nc = _get_nc()
    in_maps = _shard_inputs(Query, Key, Value, W_q, W_k, W_v, W_o)
    res = bass_utils.run_bass_kernel_spmd(nc, in_maps, core_ids=list(range(8)))
    out = np.zeros((B, S, D), dtype=np.float32)
    for b in range(B):
        for hg in range(4):
            out[b] += res.results[b * 4 + hg]["y"].astype(np.float32)
    return out



# revision 45
# speedup vs baseline: 1.0641x; 1.0048x over previous
"""Multi-head attention forward on 8 Trainium2 NeuronCores.

Sharding: core = (batch b in 0..2, head-group hg in 0..4); each core owns
4 of the 16 heads for one batch element. Q/K/V projections are computed
per-core for its 256 head-dims; attention runs per head with scores kept
transposed (S^T[k, q]); the output projection is row-sharded over W_o,
producing a per-core partial Y that the host sums over the 4 head-groups
of each batch.

v5: the kernel is PE-bound (~850 matmuls, union-busy 85% at the 87%
HAM throttle cap), with ScalarE's exp stream pacing the score ring at
1.34us/k-tile vs the PE's 1.30us: both are attacked:

- Each k-tile's even/odd-head score matmuls write one shared
  [128,2,512] PSUM slot (row-tiled tile_position (0,0)/(64,0)); both
  become ready together when the slot frees, so the PE runs the pair
  concurrently (2 matmuls per 216ns, measured).
- One exp instruction per k-tile covers both heads. Four of every 16
  k-tiles are offloaded from ScalarE to the DVE (the Pool engine
  cannot read PSUM) using a one-instruction Schraudolph exp: i16 =
  round(s*(1024/ln2)*0.125 + 15308) bit-cast as fp16 ~= exp(0.125 s)
  (max rel err ~3.5%, which largely cancels in softmax since numerator
  and denominator use the same approximation). This gives ScalarE
  ~4.7us/block of slack so the PV matmuls never pay the 100ns
  just-in-time semaphore latency on the exp results. The softmax
  normalization multiplies run on the Pool engine (all-SBUF) to make
  DVE room for the extra Schraudolph slices.
- PSUM: scores 2x[128,2,512] (4 banks), projections/output 1x[128,2,512]
  (2 banks), PV ctx accumulators 2x[128,512] (2 banks).
- Score matmuls are emitted under tc.high_priority so the PE always
  prefers feeding the exp engines; PV/projection/output work fills the
  slack. A warm-up burst of junk matmuls releases the PE HAM throttle
  before the first projection.
- Input DMAs alternate between the sync and gpsimd queue engines and
  are all dispatched up front (except the last Q slice, placed late on
  sync) so dispatch serialization never delays the pipeline and never
  blocks gpsimd's normalization broadcasts.
- PV accumulators are copied PSUM->SBUF immediately, putting the 4-hop
  normalization chain off the PSUM-reuse critical path.

All matmul operands are fp16 (PSUM accumulation stays fp32). V is stored
per k-tile in head-pair blocks [V_even | ones | junk | V_odd] (192 cols);
the PV stationary is the 128-wide window at offset 0 (even head: ctx
rows 0:64, denom row 64) or offset 64 (odd head: denom row 0, ctx rows
64:128), so each head's softmax denominator comes free. Weights are
pre-interleaved on the host so their DMAs are contiguous; output is
written fp16 and summed in f32 on the host.
"""

import sys

for _p in ("/opt/trn_rl_repo", "/opt/pypackages"):
    if _p not in sys.path:
        sys.path.append(_p)

from contextlib import ExitStack

import numpy as np

import concourse.bass as bass
import concourse.tile as tile
from concourse import bacc, mybir
from concourse import bass_utils

P = 128
B = 2
S = 2048          # sequence length
D = 1024          # model dim
H = 16            # total heads
DK = 64           # head dim
HL = 4            # heads per core
CL = HL * DK      # local head dims per core (256)
NJ = 4            # 512-wide s-slices
NS = 512
NI = D // P       # 8 contraction tiles over model dim
NK = S // P       # 16 key tiles
VPB = 192         # V pair block: V_even(64) | ones(1) | junk(63) | V_odd(64)
VPAD = 2 * VPB    # 384 cols for 2 head pairs

F32 = mybir.dt.float32
F16 = mybir.dt.float16
I16 = mybir.dt.int16
EXP = mybir.ActivationFunctionType.Exp

# k-tiles whose exp runs on the DVE (Schraudolph) instead of ScalarE
# (the Pool engine cannot read PSUM, so DVE is the only offload target);
# 4/16 along-k coverage keeps the softmax error ~1.2e-2 vs the 2e-2 gate
OFFLOAD_KS = {3, 7, 11, 15}
# i16 = s * (1024/ln2 * 0.125) + (15<<10) + C  ->  bitcast f16 ~ exp(s/8)
SCH_MULT = 184.6650
SCH_ADD = 15308.0


def build_nc():
    nc = bacc.Bacc("TRN2", target_bir_lowering=False, debug=False)

    xqT = nc.dram_tensor("xqT", [D, S], F16, kind="ExternalInput")
    xkT = nc.dram_tensor("xkT", [D, S], F16, kind="ExternalInput")
    xvT = nc.dram_tensor("xvT", [D, S], F16, kind="ExternalInput")
    # weights pre-interleaved on host to [128, n*out] layout
    wqT = nc.dram_tensor("wqT", [P, NI * CL], F16, kind="ExternalInput")
    wkT = nc.dram_tensor("wkT", [P, NI * CL], F16, kind="ExternalInput")
    wvT = nc.dram_tensor("wvT", [P, NI * CL], F16, kind="ExternalInput")
    woT = nc.dram_tensor("woT", [P, (CL // P) * D], F16, kind="ExternalInput")
    y = nc.dram_tensor("y", [S, D], F16, kind="ExternalOutput")

    _dq = [0]

    def dq():
        _dq[0] += 1
        return nc.sync if _dq[0] % 2 == 0 else nc.gpsimd

    with tile.TileContext(nc) as tc, ExitStack() as ctx:
        wpool = ctx.enter_context(tc.tile_pool(name="w", bufs=1))
        big = ctx.enter_context(tc.tile_pool(name="big", bufs=1))
        xpool = ctx.enter_context(tc.tile_pool(name="xs", bufs=26))
        epool = ctx.enter_context(tc.tile_pool(name="ex", bufs=30))
        cpool = ctx.enter_context(tc.tile_pool(name="cs", bufs=4))
        spool = ctx.enter_context(tc.tile_pool(name="sm", bufs=3))
        ypool = ctx.enter_context(tc.tile_pool(name="yo", bufs=2))
        psS = ctx.enter_context(tc.tile_pool(name="psS", bufs=2, space="PSUM"))
        psP = ctx.enter_context(tc.tile_pool(name="psP", bufs=1, space="PSUM"))
        psC = ctx.enter_context(tc.tile_pool(name="psC", bufs=2, space="PSUM"))

        wq_sb = wpool.tile([P, NI, CL], F16)
        wk_sb = wpool.tile([P, NI, CL], F16)
        wv_sb = wpool.tile([P, NI, CL], F16)
        wo_sb = wpool.tile([P, CL // P, D], F16)

        # per-slice K/Q tiles: a score matmul for k-tile k must depend
        # only on the one projection copy that wrote it, not all four
        kT_j = [big.tile([P, 2, NS], F16, tag=f"kT{j}", name=f"kT{j}")
                for j in range(NJ)]
        qT_j = [big.tile([P, 2, NS], F16, tag=f"qT{j}", name=f"qT{j}")
                for j in range(NJ)]
        cT_j = [big.tile([P, 2, NS], F16, tag=f"cT{j}", name=f"cT{j}")
                for j in range(NJ)]
        vt = [big.tile([P, VPAD], F16, tag=f"v{k}", name=f"v{k}")
              for k in range(NK)]

        # ---- PE warm-up: ~4us of junk matmuls releases the HAM gate ---
        wsrc = spool.tile([P, DK], F16, tag="wsrc", name="wsrc")
        nc.vector.memset(wsrc[:], 0.0)
        wsrc2 = spool.tile([P, NS], F16, tag="wsrc2", name="wsrc2")
        nc.vector.memset(wsrc2[:], 0.0)
        warm_ps = psC.tile([P, NS], F32, tag="ctx", name="warm")
        for _ in range(18):
            nc.tensor.matmul(warm_ps[0:DK, 0:DK], wsrc[0:DK, :],
                             wsrc[0:DK, :], start=True, stop=True)

        # ones + junk columns of each V pair block (cols 64:128, 256:320)
        for k in range(NK):
            nc.vector.memset(
                vt[k][:].rearrange("p (b c) -> p b c", c=VPB)[:, :, DK:2 * DK],
                1.0,
            )

        # ---- DMA issue helpers ----------------------------------------
        def dma_w(dst, src):
            dq().dma_start(dst[:].rearrange("p n o -> p (n o)"), src.ap())

        def dma_x(x_dram, jh, eng=None):
            xt = []
            for i in range(NI):
                t = xpool.tile([P, 2, NS], F16, tag="x", name="xt")
                (eng or dq()).dma_start(
                    t[:],
                    x_dram.ap()[i * P:(i + 1) * P, jh * 2 * NS:(jh * 2 + 2) * NS]
                    .rearrange("p (a s) -> p a s", s=NS),
                )
                xt.append(t)
            return xt

        def xti(xt, i):
            return xt[i][:]

        # ---- compute emit helpers -------------------------------------
        def proj_j(xt_jh, w_sb, jj, dst, pool=None):
            # dst <- (X @ W.T)^T for one 512-wide s-slice ([128, 2, 512]);
            # the two ot chains use independent one-bank slots so their
            # copies free PSUM independently
            if pool is None:
                spa = psP.tile([P, NS], F32, tag="ppa", name="pja")
                spb = psP.tile([P, NS], F32, tag="ppb", name="pjb")
                tgt = [spa[:], spb[:]]
            else:
                sp = pool.tile([P, 2, NS], F32, tag="sc", name="pj")
                tgt = [sp[:, 0], sp[:, 1]]
            for i in range(NI):
                for ot in range(2):
                    nc.tensor.matmul(
                        tgt[ot],
                        w_sb[:, i, ot * P:(ot + 1) * P],
                        xti(xt_jh, i)[:, jj],
                        start=(i == 0),
                        stop=(i == NI - 1),
                    )
            if pool is None:
                for ot in range(2):
                    nc.vector.tensor_copy(dst[:, ot], tgt[ot])
            else:
                nc.vector.tensor_copy(dst, sp[:])

        xv = [None, None]

        def vp(c):
            # V-proj chunk c: projects s-tiles 2c, 2c+1 and packs them
            # into vt[2c], vt[2c+1] head-pair blocks. Odd chunks borrow
            # the (still idle) ctx PSUM slots so chunk c+1's matmuls
            # overlap chunk c's pack copies instead of serializing
            # through the single projection slot.
            jh, sbp = divmod(c, 4)
            if c % 2 == 0:
                spa = psP.tile([P, NS], F32, tag="ppa", name="pva")
                spb = psP.tile([P, NS], F32, tag="ppb", name="pvb")
                tgt = [spa[:, 0:CL], spb[:, 0:CL]]
            else:
                ta = psC.tile([P, NS], F32, tag="ctx", name="pva")
                tb = psC.tile([P, NS], F32, tag="ctx", name="pvb")
                tgt = [ta[:, 0:CL], tb[:, 0:CL]]
            for i in range(NI):
                xf = xti(xv[jh], i).rearrange("p a s -> p (a s)")
                for u in range(2):
                    sb = sbp * 2 + u
                    nc.tensor.matmul(
                        tgt[u],
                        xf[:, sb * P:(sb + 1) * P],
                        wv_sb[:, i, :],
                        start=(i == 0),
                        stop=(i == NI - 1),
                    )
            for u in range(2):
                st = 2 * c + u
                vv = vt[st][:].rearrange("p (pr c) -> p pr c", c=VPB)
                pv_ = tgt[u].rearrange("p (pr hc) -> p pr hc", hc=2 * DK)
                nc.vector.tensor_copy(vv[:, :, 0:DK], pv_[:, :, 0:DK])
                nc.vector.tensor_copy(vv[:, :, 2 * DK:3 * DK], pv_[:, :, DK:2 * DK])

        pending = []

        def flush_one():
            fns = pending.pop(0)
            for fn in fns:
                fn()

        def sc_k(ot, j, k, ctx_ps, ctx_sb, off=None):
            # One k-tile: paired even/odd score matmuls into one shared
            # 2-bank slot + a single exp (ScalarE or DVE) for both heads.
            # (Splitting into per-head [128,512] slots/exps was tried and
            # regressed 212->288us: the Act engine charges ~400ns fixed
            # cost per instruction, so two half exps cost 1.66us vs 1.11.)
            sps = psS.tile([P, 2, NS], F32, tag="sc", name="sc")
            with tc.high_priority(offset=500000):
                for pr in range(2):
                    pr0 = pr * 64
                    nc.tensor.matmul(
                        sps[:, pr],
                        kT_j[k // 4][pr0:pr0 + 64, ot, (k % 4) * P:(k % 4 + 1) * P],
                        qT_j[j][pr0:pr0 + 64, ot, :],
                        start=True,
                        stop=True,
                    )
            ex = epool.tile([P, 2, NS], I16, tag="ex", name="ex")
            if k in (OFFLOAD_KS if off is None else off):
                with tc.high_priority(offset=300):
                    nc.vector.tensor_scalar(
                        ex[:], sps[:], SCH_MULT, SCH_ADD,
                        mybir.AluOpType.mult, mybir.AluOpType.add,
                    )
            else:
                nc.scalar.activation(ex[:].bitcast(F16), sps[:], EXP,
                                     scale=0.125)

            def pv_fn(ex=ex, ot=ot, k=k, first=(k == 0)):
                if first:
                    for pr in range(2):
                        ctx_ps[pr] = psC.tile([P, NS], F32, tag="ctx",
                                              name="ctx")
                exf = ex[:].bitcast(F16)
                # priority between scores and projection/output backlog:
                # late PVs hold ex-pool slots and stall the score ring
                with tc.high_priority(offset=250000):
                    for pr in range(2):
                        vcol = ot * VPB + pr * DK
                        nc.tensor.matmul(
                            ctx_ps[pr][:],
                            vt[k][:, vcol:vcol + P],
                            exf[:, pr],
                            start=(k == 0),
                            stop=(k == NK - 1),
                        )

            fns = [pv_fn]
            if k == NK - 1:
                def cp_fn():
                    # free the PSUM banks; normalize from the SBUF copy
                    for pr in range(2):
                        cs = cpool.tile([P, NS], F32, tag="cs", name="cs")
                        nc.vector.tensor_copy(cs[:], ctx_ps[pr][:])
                        ctx_sb[pr] = cs

                bcs = {}

                def norm_a():
                    # den -> recip -> gpsimd broadcast; the multiply is
                    # deferred one pending entry so the DVE stream never
                    # blocks on the broadcast (the den copy to partition 0
                    # is required: recip at a partition offset diverges on
                    # hardware even though CoreSim accepts it)
                    for pr in range(2):
                        drow = 64 * (1 - pr)
                        cs = ctx_sb[pr]
                        den = spool.tile([1, NS], F32, tag="den", name="den")
                        nc.vector.tensor_copy(den[:], cs[drow:drow + 1, :])
                        rec = spool.tile([1, NS], F32, tag="rec", name="rec")
                        nc.vector.reciprocal_approx_fast(rec[:], den[:])
                        bc = spool.tile([P, NS], F32, tag="bc", name="bc")
                        nc.gpsimd.partition_broadcast(bc[:], rec[:])
                        bcs[pr] = bc

                def norm_b(ot=ot, j=j):
                    # NOT on the Pool engine: gpsimd tensor ops live in a
                    # different microcode library than partition_broadcast,
                    # and the UNLOAD_LIB/LOAD_LIB swap between them stalls
                    # the pipeline for tens of us per block
                    for pr in range(2):
                        pr0 = pr * 64
                        nc.vector.tensor_mul(
                            cT_j[j][pr0:pr0 + 64, ot, :],
                            ctx_sb[pr][pr0:pr0 + 64, :],
                            bcs[pr][pr0:pr0 + 64, :],
                        )
                fns.extend([cp_fn, norm_a])
                pending.append(fns)
                pending.append([norm_b])
                return
            pending.append(fns)

        def emit_y_qb(j, qb, tail=False):
            # tail mode: odd chains borrow the (now idle) ctx slots so
            # the last four output chains don't serialize through the
            # single projection slot
            ysb = ypool.tile([P, D], F16, tag="y", name="ysb")
            yv = ysb[:].rearrange("p (a s) -> p a s", s=NS)
            if tail and qb % 2 == 1:
                ta = psC.tile([P, NS], F32, tag="ctx", name="ypa")
                tb = psC.tile([P, NS], F32, tag="ctx", name="ypb")
                tgt = [ta[:], tb[:]]
            else:
                ypa = psP.tile([P, NS], F32, tag="ppa", name="ya")
                ypb = psP.tile([P, NS], F32, tag="ppb", name="yb")
                tgt = [ypa[:], ypb[:]]
            for ct in range(2):
                for oh in range(2):
                    nc.tensor.matmul(
                        tgt[oh],
                        cT_j[j][:, ct, qb * P:(qb + 1) * P],
                        wo_sb[:, ct, oh * NS:(oh + 1) * NS],
                        start=(ct == 0),
                        stop=(ct == 1),
                    )
            for oh in range(2):
                nc.vector.tensor_copy(yv[:, oh], tgt[oh])
            nc.sync.dma_start(
                y.ap()[(j * 4 + qb) * P:(j * 4 + qb + 1) * P, :], ysb[:]
            )

        def emit_block(ot, j, lag=3, weave_y=None):
            # weave_y: output chains for q-slice weave_y are interleaved
            # into this block's k-loop so they overlap the attention
            # stream instead of serializing at the kernel tail (their cT
            # inputs are two block-norms old by then)
            ctx_ps, ctx_sb = {}, {}
            for k in range(NK):
                sc_k(ot, j, k, ctx_ps, ctx_sb)
                while lag is not None and len(pending) > lag:
                    flush_one()
                # y chains woven 4 tiles apart (after this tile's PV
                # flush) so each qb's psP banks are long free when the
                # next qb needs them and the Act engine keeps receiving
                # scores at a steady cadence instead of burst-then-idle
                if weave_y is not None and k in (2, 6, 10, 14):
                    emit_y_qb(weave_y, (k - 2) // 4)

        def emit_y(j, tail=False):
            for qb in range(4):
                emit_y_qb(j, qb, tail)

        # ---- pipelined schedule ---------------------------------------
        dma_w(wq_sb, wqT)
        dma_w(wk_sb, wkT)
        xq0 = dma_x(xqT, 0)
        xk0 = dma_x(xkT, 0)
        xk1 = dma_x(xkT, 1)
        dma_w(wv_sb, wvT)
        xv[0] = dma_x(xvT, 0)
        dma_w(wo_sb, woT)
        xv[1] = dma_x(xvT, 1)

        # first Q/K chains borrow the (still idle) score slots so the
        # single psP slot doesn't serialize the pipeline start
        # second warm-up burst first in line: N=512 junk matmuls span the
        # input-DMA wait so the HAM MID window never sees a >3.4us idle
        warm2 = psC.tile([P, NS], F32, tag="ctx", name="warm2")
        for _ in range(13):
            nc.tensor.matmul(warm2[0:DK, :], wsrc[0:DK, :],
                             wsrc2[0:DK, :], start=True, stop=True)

        proj_j(xq0, wq_sb, 0, qT_j[0][:, :, :], pool=psS)
        proj_j(xk0, wk_sb, 0, kT_j[0][:, :, :], pool=psS)
        proj_j(xk0, wk_sb, 1, kT_j[1][:, :, :])
        proj_j(xk1, wk_sb, 0, kT_j[2][:, :, :])
        proj_j(xk1, wk_sb, 1, kT_j[3][:, :, :])

        # block (0,0) fills the pipeline with nothing else for ScalarE to
        # overlap, so split its exps 50/50 with the (idle) DVE; the extra
        # Schraudolph tiles on 2 of 64 block-instances are negligible
        b00_ctx, b00_cs = {}, {}
        for k in range(NK):
            sc_k(0, 0, k, b00_ctx, b00_cs,      # no flush: V not built yet
                 off={1, 3, 5, 7, 9, 11, 13, 15})

        proj_j(xq0, wq_sb, 1, qT_j[1][:, :, :])

        # block (1,0) woven with V-proj chunks; flush (0,0) PV entries as
        # soon as the vt tiles they read exist (entry m needs vp(m//2)).
        b10_ctx, b10_cs = {}, {}
        for k in range(NK):
            sc_k(1, 0, k, b10_ctx, b10_cs)
            if k < 8:
                vp(k)
        # flush (0,0) PVs only now: the odd V chunks borrow ctx PSUM
        # slots, so ctx allocations must follow all vp allocations
        for _ in range(16):
            flush_one()

        xq1 = dma_x(xqT, 1, eng=nc.sync)        # late; sync is free now
        proj_j(xq1, wq_sb, 0, qT_j[2][:, :, :])
        proj_j(xq1, wq_sb, 1, qT_j[3][:, :, :])

        emit_block(0, 1)
        emit_block(1, 1)
        emit_block(0, 2, weave_y=0)
        emit_block(1, 2)
        emit_block(0, 3, weave_y=1)
        emit_block(1, 3, weave_y=2)
        while pending:
            flush_one()
        # keep-hot junk: the final norm chain leaves the PE idle for ~2us,
        # which drops it to the MID p-state and makes the last 16 output
        # matmuls run ~3x slow; these fillers keep the clock up
        warm3 = psS.tile([P, 2, NS], F32, tag="sc", name="warm3")
        for _ in range(6):
            nc.tensor.matmul(warm3[0:DK, 0], wsrc[0:DK, :],
                             wsrc2[0:DK, :], start=True, stop=True)
        emit_y(3, tail=True)

    nc.compile()
    return nc


_NC = None


def _get_nc():
    global _NC
    if _NC is None:
        _NC = build_nc()
    return _NC


def _interleave_w(w):
    # [NI*P, O] -> [P, NI*O] so the SBUF load DMA is contiguous
    n = w.shape[0] // P
    return np.ascontiguousarray(
        w.reshape(n, P, w.shape[1]).transpose(1, 0, 2).reshape(P, -1)
    ).astype(np.float16)


def _shard_inputs(Query, Key, Value, W_q, W_k, W_v, W_o):
    in_maps = []
    xT = {}
    for b in range(B):
        xT[b] = (
            np.ascontiguousarray(Query[b].T).astype(np.float16),
            np.ascontiguousarray(Key[b].T).astype(np.float16),
            np.ascontiguousarray(Value[b].T).astype(np.float16),
        )
    for b in range(B):
        for hg in range(4):
            r0 = hg * CL
            in_maps.append({
                "xqT": xT[b][0],
                "xkT": xT[b][1],
                "xvT": xT[b][2],
                "wqT": _interleave_w(np.ascontiguousarray(W_q[r0:r0 + CL, :].T)),
                "wkT": _interleave_w(np.ascontiguousarray(W_k[r0:r0 + CL, :].T)),
                "wvT": _interleave_w(np.ascontiguousarray(W_v[r0:r0 + CL, :].T)),
                "woT": _interleave_w(np.ascontiguousarray(W_o[:, r0:r0 + CL].T)),
            })
    return in_maps


def _reference_np(Query, Key, Value, mask, W_q, W_k, W_v, W_o):
    # Fallback for a non-trivial mask (never hit for the spec'd inputs).
    out = np.empty((B, S, D), dtype=np.float32)
    m = np.broadcast_to(mask, (1, 1, S, S))[0, 0]
    for b in range(B):
        Q = (Query[b] @ W_q.T).reshape(S, H, DK).transpose(1, 0, 2)
        K = (Key[b] @ W_k.T).reshape(S, H, DK).transpose(1, 0, 2)
        V = (Value[b] @ W_v.T).reshape(S, H, DK).transpose(1, 0, 2)
        ctx = np.empty((H, S, DK), dtype=np.float32)
        for h in range(H):
            s = (Q[h] @ K[h].T) / np.sqrt(DK)
            s = np.where(m == 0, -1e9, s)
            s -= s.max(axis=-1, keepdims=True)
            e = np.exp(s)
            ctx[h] = (e / e.sum(axis=-1, keepdims=True)) @ V[h]
        out[b] = ctx.transpose(1, 0, 2).reshape(S, D) @ W_o.T
    return out


def kernel(Query, Key, Value, mask, W_q, W_k, W_v, W_o, **_ignored):
    Query = np.asarray(Query, dtype=np.float32)
    Key = np.asarray(Key, dtype=np.float32)
    Value = np.asarray(Value, dtype=np.float32)
    W_q = np.asarray(W_q, dtype=np.float32)
    W_k = np.asarray(W_k, dtype=np.float32)
    W_v = np.asarray(W_v, dtype=np.float32)
    W_o = np.asarray(W_o, dtype=np.float32)

    if not np.all(np.asarray(mask) != 0):
        return _reference_np(Query, Key, Value, np.asarray(mask),
                             W_q, W_k, W_v, W_o)

    nc = _get_nc()
    in_maps = _shard_inputs(Query, Key, Value, W_q, W_k, W_v, W_o)
    res = bass_utils.run_bass_kernel_spmd(nc, in_maps, core_ids=list(range(8)))
    out = np.zeros((B, S, D), dtype=np.float32)
    for b in range(B):
        for hg in range(4):
            out[b] += res.results[b * 4 + hg]["y"].astype(np.float32)
    return out



# revision 47
# speedup vs baseline: 1.0652x; 1.0010x over previous
"""Multi-head attention forward on 8 Trainium2 NeuronCores.

Sharding: core = (batch b in 0..2, head-group hg in 0..4); each core owns
4 of the 16 heads for one batch element. Q/K/V projections are computed
per-core for its 256 head-dims; attention runs per head with scores kept
transposed (S^T[k, q]); the output projection is row-sharded over W_o,
producing a per-core partial Y that the host sums over the 4 head-groups
of each batch.

v12 (250us -> 210us): the kernel is PE-bound (~870 matmuls, 84%
union-busy), with ScalarE's 1.1us/k-tile exp stream as the secondary
constraint:

- Each k-tile's even/odd-head score matmuls write one shared
  [128,2,512] PSUM slot (row-tiled tile_position (0,0)/(64,0)); both
  become ready together when the slot frees, so the PE runs the pair
  concurrently. Keeping the pair + a single exp per k-tile matters:
  the Act engine charges ~400ns fixed cost per instruction, so
  per-head [128,512] exps cost 1.66us/tile vs 1.11 (tried, 288us).
- Four of every 16 k-tiles ({3,7,11,15}, so the fast path frees the
  score ring at block seams) are offloaded from ScalarE to the DVE
  (the Pool engine cannot read PSUM) using a one-instruction
  Schraudolph exp: i16 = round(s*(1024/ln2)*0.125 + 15308) bit-cast as
  fp16 ~= exp(0.125 s) (max rel err ~3.5%, which largely cancels in
  softmax since numerator and denominator share the approximation;
  along-k coverage of 25% keeps end-to-end error ~1.2e-2 vs the 2e-2
  gate). Block (0,0) splits 8/16 to the then-idle DVE to shorten the
  pipeline-fill serial exp chain. 6/16 steady-state offload regressed.
- PSUM: scores 2x[128,2,512] (4 banks), projection/output chains two
  independent one-bank [128,512] slots (so the two halves of a chain
  free PSUM independently and back-to-back output chains pipeline),
  PV ctx accumulators 2x[128,512] (2 banks).
- Score matmuls are emitted under tc.high_priority so the PE always
  prefers feeding the exp engines (lowering them below the PVs
  regressed: the exp stream is still the ring's heartbeat);
  PV/projection/output work fills the slack.
- Output chains for q-slice j are woven into block (0,j+2)'s k-loop
  four tiles apart, overlapping the attention stream instead of
  serializing at the kernel tail; the last slice runs after a burst of
  keep-hot junk matmuls that stops the HAM throttle from halving the
  PE clock during the final norm-chain wait (HAM duty drops after any
  PE idle; warm-up junk at the start serves the same purpose).
- Input DMAs alternate between the sync and gpsimd queue engines and
  are all dispatched up front (except the last Q slice, placed late on
  sync) so dispatch serialization never delays the pipeline and never
  blocks gpsimd's normalization broadcasts.
- PV accumulators are copied PSUM->SBUF immediately, putting the 4-hop
  normalization chain off the PSUM-reuse critical path. The denom row
  must be copied to partition 0 before reciprocal_approx_fast: the
  custom DVE op misbehaves on hardware with a partition-offset input
  (CoreSim accepts it). Norm multiplies stay on the DVE: gpsimd
  tensor ops live in a different ucode library than
  partition_broadcast and the UNLOAD/LOAD_LIB swap costs tens of us.

All matmul operands are fp16 (PSUM accumulation stays fp32). V is stored
per k-tile in head-pair blocks [V_even | ones | junk | V_odd] (192 cols);
the PV stationary is the 128-wide window at offset 0 (even head: ctx
rows 0:64, denom row 64) or offset 64 (odd head: denom row 0, ctx rows
64:128), so each head's softmax denominator comes free. Weights are
pre-interleaved on the host so their DMAs are contiguous; output is
written fp16 and summed in f32 on the host.
"""

import sys

for _p in ("/opt/trn_rl_repo", "/opt/pypackages"):
    if _p not in sys.path:
        sys.path.append(_p)

from contextlib import ExitStack

import numpy as np

import concourse.bass as bass
import concourse.tile as tile
from concourse import bacc, mybir
from concourse import bass_utils

P = 128
B = 2
S = 2048          # sequence length
D = 1024          # model dim
H = 16            # total heads
DK = 64           # head dim
HL = 4            # heads per core
CL = HL * DK      # local head dims per core (256)
NJ = 4            # 512-wide s-slices
NS = 512
NI = D // P       # 8 contraction tiles over model dim
NK = S // P       # 16 key tiles
VPB = 192         # V pair block: V_even(64) | ones(1) | junk(63) | V_odd(64)
VPAD = 2 * VPB    # 384 cols for 2 head pairs

F32 = mybir.dt.float32
F16 = mybir.dt.float16
I16 = mybir.dt.int16
EXP = mybir.ActivationFunctionType.Exp

# k-tiles whose exp runs on the DVE (Schraudolph) instead of ScalarE
# (the Pool engine cannot read PSUM, so DVE is the only offload target);
# 4/16 along-k coverage keeps the softmax error ~1.2e-2 vs the 2e-2 gate
OFFLOAD_KS = {3, 7, 11, 15}
# i16 = s * (1024/ln2 * 0.125) + (15<<10) + C  ->  bitcast f16 ~ exp(s/8)
SCH_MULT = 184.6650
SCH_ADD = 15308.0


def build_nc():
    nc = bacc.Bacc("TRN2", target_bir_lowering=False, debug=False)

    xqT = nc.dram_tensor("xqT", [D, S], F16, kind="ExternalInput")
    xkT = nc.dram_tensor("xkT", [D, S], F16, kind="ExternalInput")
    xvT = nc.dram_tensor("xvT", [D, S], F16, kind="ExternalInput")
    # weights pre-interleaved on host to [128, n*out] layout
    wqT = nc.dram_tensor("wqT", [P, NI * CL], F16, kind="ExternalInput")
    wkT = nc.dram_tensor("wkT", [P, NI * CL], F16, kind="ExternalInput")
    wvT = nc.dram_tensor("wvT", [P, NI * CL], F16, kind="ExternalInput")
    woT = nc.dram_tensor("woT", [P, (CL // P) * D], F16, kind="ExternalInput")
    y = nc.dram_tensor("y", [S, D], F16, kind="ExternalOutput")

    _dq = [0]

    def dq():
        _dq[0] += 1
        return nc.sync if _dq[0] % 2 == 0 else nc.gpsimd

    with tile.TileContext(nc) as tc, ExitStack() as ctx:
        wpool = ctx.enter_context(tc.tile_pool(name="w", bufs=1))
        big = ctx.enter_context(tc.tile_pool(name="big", bufs=1))
        xpool = ctx.enter_context(tc.tile_pool(name="xs", bufs=26))
        epool = ctx.enter_context(tc.tile_pool(name="ex", bufs=30))
        cpool = ctx.enter_context(tc.tile_pool(name="cs", bufs=4))
        spool = ctx.enter_context(tc.tile_pool(name="sm", bufs=3))
        ypool = ctx.enter_context(tc.tile_pool(name="yo", bufs=2))
        psS = ctx.enter_context(tc.tile_pool(name="psS", bufs=2, space="PSUM"))
        psP = ctx.enter_context(tc.tile_pool(name="psP", bufs=1, space="PSUM"))
        psC = ctx.enter_context(tc.tile_pool(name="psC", bufs=2, space="PSUM"))

        wq_sb = wpool.tile([P, NI, CL], F16)
        wk_sb = wpool.tile([P, NI, CL], F16)
        wv_sb = wpool.tile([P, NI, CL], F16)
        wo_sb = wpool.tile([P, CL // P, D], F16)

        # per-slice K/Q tiles: a score matmul for k-tile k must depend
        # only on the one projection copy that wrote it, not all four
        kT_j = [big.tile([P, 2, NS], F16, tag=f"kT{j}", name=f"kT{j}")
                for j in range(NJ)]
        qT_j = [big.tile([P, 2, NS], F16, tag=f"qT{j}", name=f"qT{j}")
                for j in range(NJ)]
        cT_j = [big.tile([P, 2, NS], F16, tag=f"cT{j}", name=f"cT{j}")
                for j in range(NJ)]
        vt = [big.tile([P, VPAD], F16, tag=f"v{k}", name=f"v{k}")
              for k in range(NK)]

        # ---- PE warm-up: ~4us of junk matmuls releases the HAM gate ---
        wsrc = spool.tile([P, DK], F16, tag="wsrc", name="wsrc")
        nc.vector.memset(wsrc[:], 0.0)
        wsrc2 = spool.tile([P, NS], F16, tag="wsrc2", name="wsrc2")
        nc.vector.memset(wsrc2[:], 0.0)
        warm_ps = psC.tile([P, NS], F32, tag="ctx", name="warm")
        for _ in range(18):
            nc.tensor.matmul(warm_ps[0:DK, 0:DK], wsrc[0:DK, :],
                             wsrc[0:DK, :], start=True, stop=True)

        # ones + junk columns of each V pair block (cols 64:128, 256:320)
        for k in range(NK):
            nc.vector.memset(
                vt[k][:].rearrange("p (b c) -> p b c", c=VPB)[:, :, DK:2 * DK],
                1.0,
            )

        # ---- DMA issue helpers ----------------------------------------
        def dma_w(dst, src):
            dq().dma_start(dst[:].rearrange("p n o -> p (n o)"), src.ap())

        def dma_x(x_dram, jh, eng=None):
            xt = []
            for i in range(NI):
                t = xpool.tile([P, 2, NS], F16, tag="x", name="xt")
                (eng or dq()).dma_start(
                    t[:],
                    x_dram.ap()[i * P:(i + 1) * P, jh * 2 * NS:(jh * 2 + 2) * NS]
                    .rearrange("p (a s) -> p a s", s=NS),
                )
                xt.append(t)
            return xt

        def xti(xt, i):
            return xt[i][:]

        # ---- compute emit helpers -------------------------------------
        def proj_j(xt_jh, w_sb, jj, dst, pool=None):
            # dst <- (X @ W.T)^T for one 512-wide s-slice ([128, 2, 512]);
            # the two ot chains use independent one-bank slots so their
            # copies free PSUM independently
            if pool is None:
                spa = psP.tile([P, NS], F32, tag="ppa", name="pja")
                spb = psP.tile([P, NS], F32, tag="ppb", name="pjb")
                tgt = [spa[:], spb[:]]
            else:
                sp = pool.tile([P, 2, NS], F32, tag="sc", name="pj")
                tgt = [sp[:, 0], sp[:, 1]]
            for i in range(NI):
                for ot in range(2):
                    nc.tensor.matmul(
                        tgt[ot],
                        w_sb[:, i, ot * P:(ot + 1) * P],
                        xti(xt_jh, i)[:, jj],
                        start=(i == 0),
                        stop=(i == NI - 1),
                    )
            if pool is None:
                for ot in range(2):
                    nc.vector.tensor_copy(dst[:, ot], tgt[ot])
            else:
                nc.vector.tensor_copy(dst, sp[:])

        xv = [None, None]

        def vp(c):
            # V-proj chunk c: projects s-tiles 2c, 2c+1 and packs them
            # into vt[2c], vt[2c+1] head-pair blocks. Odd chunks borrow
            # the (still idle) ctx PSUM slots so chunk c+1's matmuls
            # overlap chunk c's pack copies instead of serializing
            # through the single projection slot.
            jh, sbp = divmod(c, 4)
            if c % 2 == 0:
                spa = psP.tile([P, NS], F32, tag="ppa", name="pva")
                spb = psP.tile([P, NS], F32, tag="ppb", name="pvb")
                tgt = [spa[:, 0:CL], spb[:, 0:CL]]
            else:
                ta = psC.tile([P, NS], F32, tag="ctx", name="pva")
                tb = psC.tile([P, NS], F32, tag="ctx", name="pvb")
                tgt = [ta[:, 0:CL], tb[:, 0:CL]]
            for i in range(NI):
                xf = xti(xv[jh], i).rearrange("p a s -> p (a s)")
                for u in range(2):
                    sb = sbp * 2 + u
                    nc.tensor.matmul(
                        tgt[u],
                        xf[:, sb * P:(sb + 1) * P],
                        wv_sb[:, i, :],
                        start=(i == 0),
                        stop=(i == NI - 1),
                    )
            for u in range(2):
                st = 2 * c + u
                vv = vt[st][:].rearrange("p (pr c) -> p pr c", c=VPB)
                pv_ = tgt[u].rearrange("p (pr hc) -> p pr hc", hc=2 * DK)
                nc.vector.tensor_copy(vv[:, :, 0:DK], pv_[:, :, 0:DK])
                nc.vector.tensor_copy(vv[:, :, 2 * DK:3 * DK], pv_[:, :, DK:2 * DK])

        pending = []

        def flush_one():
            fns = pending.pop(0)
            for fn in fns:
                fn()

        def sc_k(ot, j, k, ctx_ps, ctx_sb, off=None):
            # One k-tile: paired even/odd score matmuls into one shared
            # 2-bank slot + a single exp (ScalarE or DVE) for both heads.
            # (Splitting into per-head [128,512] slots/exps was tried and
            # regressed 212->288us: the Act engine charges ~400ns fixed
            # cost per instruction, so two half exps cost 1.66us vs 1.11.)
            sps = psS.tile([P, 2, NS], F32, tag="sc", name="sc")
            with tc.high_priority(offset=500000):
                for pr in range(2):
                    pr0 = pr * 64
                    nc.tensor.matmul(
                        sps[:, pr],
                        kT_j[k // 4][pr0:pr0 + 64, ot, (k % 4) * P:(k % 4 + 1) * P],
                        qT_j[j][pr0:pr0 + 64, ot, :],
                        start=True,
                        stop=True,
                    )
            ex = epool.tile([P, 2, NS], I16, tag="ex", name="ex")
            if k in (OFFLOAD_KS if off is None else off):
                with tc.high_priority(offset=300):
                    nc.vector.tensor_scalar(
                        ex[:], sps[:], SCH_MULT, SCH_ADD,
                        mybir.AluOpType.mult, mybir.AluOpType.add,
                    )
            else:
                nc.scalar.activation(ex[:].bitcast(F16), sps[:], EXP,
                                     scale=0.125)

            def pv_fn(ex=ex, ot=ot, k=k, first=(k == 0)):
                if first:
                    for pr in range(2):
                        ctx_ps[pr] = psC.tile([P, NS], F32, tag="ctx",
                                              name="ctx")
                exf = ex[:].bitcast(F16)
                # priority between scores and projection/output backlog:
                # late PVs hold ex-pool slots and stall the score ring
                with tc.high_priority(offset=250000):
                    for pr in range(2):
                        vcol = ot * VPB + pr * DK
                        nc.tensor.matmul(
                            ctx_ps[pr][:],
                            vt[k][:, vcol:vcol + P],
                            exf[:, pr],
                            start=(k == 0),
                            stop=(k == NK - 1),
                        )

            fns = [pv_fn]
            if k == NK - 1:
                def cp_fn():
                    # free the PSUM banks; normalize from the SBUF copy
                    for pr in range(2):
                        cs = cpool.tile([P, NS], F32, tag="cs", name="cs")
                        nc.vector.tensor_copy(cs[:], ctx_ps[pr][:])
                        ctx_sb[pr] = cs

                bcs = {}

                def norm_a():
                    # den -> recip -> gpsimd broadcast; the multiply is
                    # deferred one pending entry so the DVE stream never
                    # blocks on the broadcast (the den copy to partition 0
                    # is required: recip at a partition offset diverges on
                    # hardware even though CoreSim accepts it)
                    for pr in range(2):
                        drow = 64 * (1 - pr)
                        cs = ctx_sb[pr]
                        den = spool.tile([1, NS], F32, tag="den", name="den")
                        nc.vector.tensor_copy(den[:], cs[drow:drow + 1, :])
                        rec = spool.tile([1, NS], F32, tag="rec", name="rec")
                        nc.vector.reciprocal_approx_fast(rec[:], den[:])
                        bc = spool.tile([P, NS], F32, tag="bc", name="bc")
                        nc.gpsimd.partition_broadcast(bc[:], rec[:])
                        bcs[pr] = bc

                def norm_b(ot=ot, j=j):
                    # NOT on the Pool engine: gpsimd tensor ops live in a
                    # different microcode library than partition_broadcast,
                    # and the UNLOAD_LIB/LOAD_LIB swap between them stalls
                    # the pipeline for tens of us per block
                    for pr in range(2):
                        pr0 = pr * 64
                        nc.vector.tensor_mul(
                            cT_j[j][pr0:pr0 + 64, ot, :],
                            ctx_sb[pr][pr0:pr0 + 64, :],
                            bcs[pr][pr0:pr0 + 64, :],
                        )
                fns.extend([cp_fn, norm_a])
                pending.append(fns)
                pending.append([norm_b])
                return
            pending.append(fns)

        def emit_y_qb(j, qb, tail=False):
            # tail mode: odd chains borrow the (now idle) ctx slots so
            # the last four output chains don't serialize through the
            # single projection slot
            ysb = ypool.tile([P, D], F16, tag="y", name="ysb")
            yv = ysb[:].rearrange("p (a s) -> p a s", s=NS)
            if tail and qb % 2 == 1:
                ta = psC.tile([P, NS], F32, tag="ctx", name="ypa")
                tb = psC.tile([P, NS], F32, tag="ctx", name="ypb")
                tgt = [ta[:], tb[:]]
            else:
                ypa = psP.tile([P, NS], F32, tag="ppa", name="ya")
                ypb = psP.tile([P, NS], F32, tag="ppb", name="yb")
                tgt = [ypa[:], ypb[:]]
            for ct in range(2):
                for oh in range(2):
                    nc.tensor.matmul(
                        tgt[oh],
                        cT_j[j][:, ct, qb * P:(qb + 1) * P],
                        wo_sb[:, ct, oh * NS:(oh + 1) * NS],
                        start=(ct == 0),
                        stop=(ct == 1),
                    )
            for oh in range(2):
                nc.vector.tensor_copy(yv[:, oh], tgt[oh])
            nc.sync.dma_start(
                y.ap()[(j * 4 + qb) * P:(j * 4 + qb + 1) * P, :], ysb[:]
            )

        def emit_block(ot, j, lag=4, weave_y=None):
            # weave_y: output chains for q-slice weave_y are interleaved
            # into this block's k-loop so they overlap the attention
            # stream instead of serializing at the kernel tail (their cT
            # inputs are two block-norms old by then)
            ctx_ps, ctx_sb = {}, {}
            for k in range(NK):
                sc_k(ot, j, k, ctx_ps, ctx_sb)
                while lag is not None and len(pending) > lag:
                    flush_one()
                # y chains woven 4 tiles apart (after this tile's PV
                # flush) so each qb's psP banks are long free when the
                # next qb needs them and the Act engine keeps receiving
                # scores at a steady cadence instead of burst-then-idle
                if weave_y is not None and k in (2, 6, 10, 14):
                    emit_y_qb(weave_y, (k - 2) // 4)

        def emit_y(j, tail=False):
            for qb in range(4):
                emit_y_qb(j, qb, tail)

        # ---- pipelined schedule ---------------------------------------
        dma_w(wq_sb, wqT)
        dma_w(wk_sb, wkT)
        xq0 = dma_x(xqT, 0)
        xk0 = dma_x(xkT, 0)
        xk1 = dma_x(xkT, 1)
        dma_w(wv_sb, wvT)
        xv[0] = dma_x(xvT, 0)
        dma_w(wo_sb, woT)
        xv[1] = dma_x(xvT, 1)

        # first Q/K chains borrow the (still idle) score slots so the
        # single psP slot doesn't serialize the pipeline start
        # second warm-up burst first in line: N=512 junk matmuls span the
        # input-DMA wait so the HAM MID window never sees a >3.4us idle
        warm2 = psC.tile([P, NS], F32, tag="ctx", name="warm2")
        for _ in range(13):
            nc.tensor.matmul(warm2[0:DK, :], wsrc[0:DK, :],
                             wsrc2[0:DK, :], start=True, stop=True)

        proj_j(xq0, wq_sb, 0, qT_j[0][:, :, :], pool=psS)
        proj_j(xk0, wk_sb, 0, kT_j[0][:, :, :], pool=psS)
        proj_j(xk0, wk_sb, 1, kT_j[1][:, :, :])
        proj_j(xk1, wk_sb, 0, kT_j[2][:, :, :])
        proj_j(xk1, wk_sb, 1, kT_j[3][:, :, :])

        # block (0,0) fills the pipeline with nothing else for ScalarE to
        # overlap, so split its exps 50/50 with the (idle) DVE; the extra
        # Schraudolph tiles on 2 of 64 block-instances are negligible
        b00_ctx, b00_cs = {}, {}
        for k in range(NK):
            sc_k(0, 0, k, b00_ctx, b00_cs,      # no flush: V not built yet
                 off={1, 3, 5, 7, 9, 11, 13, 15})

        proj_j(xq0, wq_sb, 1, qT_j[1][:, :, :])

        # block (1,0) woven with V-proj chunks; flush (0,0) PV entries as
        # soon as the vt tiles they read exist (entry m needs vp(m//2)).
        b10_ctx, b10_cs = {}, {}
        for k in range(NK):
            sc_k(1, 0, k, b10_ctx, b10_cs)
            if k < 8:
                vp(k)
        # flush (0,0) PVs only now: the odd V chunks borrow ctx PSUM
        # slots, so ctx allocations must follow all vp allocations
        for _ in range(16):
            flush_one()

        xq1 = dma_x(xqT, 1, eng=nc.sync)        # late; sync is free now
        proj_j(xq1, wq_sb, 0, qT_j[2][:, :, :])
        proj_j(xq1, wq_sb, 1, qT_j[3][:, :, :])

        emit_block(0, 1)
        emit_block(1, 1)
        emit_block(0, 2, weave_y=0)
        emit_block(1, 2)
        emit_block(0, 3, weave_y=1)
        emit_block(1, 3, weave_y=2)
        while pending:
            flush_one()
        # keep-hot junk: the final norm chain leaves the PE idle for ~2us,
        # which drops it to the MID p-state and makes the last 16 output
        # matmuls run ~3x slow; these fillers keep the clock up
        warm3 = psS.tile([P, 2, NS], F32, tag="sc", name="warm3")
        for _ in range(6):
            nc.tensor.matmul(warm3[0:DK, 0], wsrc[0:DK, :],
                             wsrc2[0:DK, :], start=True, stop=True)
        emit_y(3, tail=True)

    nc.compile()
    return nc


_NC = None


def _get_nc():
    global _NC
    if _NC is None:
        _NC = build_nc()
    return _NC


def _interleave_w(w):
    # [NI*P, O] -> [P, NI*O] so the SBUF load DMA is contiguous
    n = w.shape[0] // P
    return np.ascontiguousarray(
        w.reshape(n, P, w.shape[1]).transpose(1, 0, 2).reshape(P, -1)
    ).astype(np.float16)


def _shard_inputs(Query, Key, Value, W_q, W_k, W_v, W_o):
    in_maps = []
    xT = {}
    for b in range(B):
        xT[b] = (
            np.ascontiguousarray(Query[b].T).astype(np.float16),
            np.ascontiguousarray(Key[b].T).astype(np.float16),
            np.ascontiguousarray(Value[b].T).astype(np.float16),
        )
    for b in range(B):
        for hg in range(4):
            r0 = hg * CL
            in_maps.append({
                "xqT": xT[b][0],
                "xkT": xT[b][1],
                "xvT": xT[b][2],
                "wqT": _interleave_w(np.ascontiguousarray(W_q[r0:r0 + CL, :].T)),
                "wkT": _interleave_w(np.ascontiguousarray(W_k[r0:r0 + CL, :].T)),
                "wvT": _interleave_w(np.ascontiguousarray(W_v[r0:r0 + CL, :].T)),
                "woT": _interleave_w(np.ascontiguousarray(W_o[:, r0:r0 + CL].T)),
            })
    return in_maps


def _reference_np(Query, Key, Value, mask, W_q, W_k, W_v, W_o):
    # Fallback for a non-trivial mask (never hit for the spec'd inputs).
    out = np.empty((B, S, D), dtype=np.float32)
    m = np.broadcast_to(mask, (1, 1, S, S))[0, 0]
    for b in range(B):
        Q = (Query[b] @ W_q.T).reshape(S, H, DK).transpose(1, 0, 2)
        K = (Key[b] @ W_k.T).reshape(S, H, DK).transpose(1, 0, 2)
        V = (Value[b] @ W_v.T).reshape(S, H, DK).transpose(1, 0, 2)
        ctx = np.empty((H, S, DK), dtype=np.float32)
        for h in range(H):
            s = (Q[h] @ K[h].T) / np.sqrt(DK)
            s = np.where(m == 0, -1e9, s)
            s -= s.max(axis=-1, keepdims=True)
            e = np.exp(s)
            ctx[h] = (e / e.sum(axis=-1, keepdims=True)) @ V[h]
        out[b] = ctx.transpose(1, 0, 2).reshape(S, D) @ W_o.T
    return out


def kernel(Query, Key, Value, mask, W_q, W_k, W_v, W_o, **_ignored):
    Query = np.asarray(Query, dtype=np.float32)
    Key = np.asarray(Key, dtype=np.float32)
    Value = np.asarray(Value, dtype=np.float32)
    W_q = np.asarray(W_q, dtype=np.float32)
    W_k = np.asarray(W_k, dtype=np.float32)
    W_v = np.asarray(W_v, dtype=np.float32)
    W_o = np.asarray(W_o, dtype=np.float32)

    if not np.all(np.asarray(mask) != 0):
        return _reference_np(Query, Key, Value, np.asarray(mask),
                             W_q, W_k, W_v, W_o)

    nc = _get_nc()
    in_maps = _shard_inputs(Query, Key, Value, W_q, W_k, W_v, W_o)
    res = bass_utils.run_bass_kernel_spmd(nc, in_maps, core_ids=list(range(8)))
    out = np.zeros((B, S, D), dtype=np.float32)
    for b in range(B):
        for hg in range(4):
            out[b] += res.results[b * 4 + hg]["y"].astype(np.float32)
    return out

